# revision 4
# baseline (speedup 1.0000x reference)
"""DescriptorLoss kernel for Trainium2 (8 NeuronCores, SPMD data-parallel).

Math:
    d[b,ij,kl] = sum_c desc0[b,c,ij] * desc1[b,c,kl]
    loss = mean(where(mask, 250*relu(1 - d), relu(d - 0.2)))

Device strategy (per core, 8 cores, shard = (batch, i-slab)):
    d' = 5*d computed as bf16 matmul into PSUM fp32 (lhsT = 5*desc0 slab).
    relu(d-0.2)   = (max(d',1) - 1)/5     hinge at 1 (exact in fp8)
    relu(1-d)     = (5 - min(d',5))/5     hinge at 5 (exact in fp8)
    Masked sums via ONE fused DVE scalar_tensor_tensor op each:
      r1 = min(max(d',1), X)  X = 1    if m else 2048   -> acc1 += sum(r1)
      r2 = max(min(d',5), Y)  Y = -2048 if m else 5     -> acc2 += sum(r2)
    With N elements per core:
      sum((1-m)*relu(d-0.2)) = (acc1 - N)/5
      sum(m*relu(1-d))       = (N - acc2/5)  =>  (5N - acc2)/5
      S_core = (acc1 - 250*acc2 + 1249*N)/5
    loss = sum_cores(S_core) / (B*H*W*H*W)
All clamp constants (1, 5, +-2048) are exact in float8_e5m2, so the
encodings introduce zero error beyond the bf16 matmul rounding.
"""

import numpy as np
import ml_dtypes

import concourse.bass as bass
import concourse.bacc as bacc
import concourse.mybir as mybir
import concourse.tile as tile
from concourse.bass_utils import run_bass_kernel_spmd

B, D, H, W = 2, 128, 64, 64
N_CORES = 8
IJ = H * W              # 4096
ROWS_PER_CORE = IJ // 4  # 1024 (i-slab of 16 rows x 64 j)
N_PER_CORE = ROWS_PER_CORE * IJ  # 4096*1024
G = ROWS_PER_CORE // 128  # 8 ij-chunks of 128 rows
KTILE = 1024              # kl columns per STT tile
KT = IJ // KTILE          # 4 kl-chunks

CLAMP = 2048.0  # exact in fp8e5m2; |d'| hard-bounded ~1100

_cached = {}


def _build_program():
    nc = bacc.Bacc("TRN2")
    f32 = mybir.dt.float32
    bf16 = mybir.dt.bfloat16
    f8 = mybir.dt.float8e5

    a5 = nc.declare_dram_parameter("a5", [D, ROWS_PER_CORE], bf16, isOutput=False)
    bm = nc.declare_dram_parameter("bm", [D, IJ], bf16, isOutput=False)
    xm = nc.declare_dram_parameter("xm", [ROWS_PER_CORE, IJ], f8, isOutput=False)
    ym = nc.declare_dram_parameter("ym", [ROWS_PER_CORE, IJ], f8, isOutput=False)
    acc1_out = nc.declare_dram_parameter("acc1", [128, G * KT], f32, isOutput=True)
    acc2_out = nc.declare_dram_parameter("acc2", [128, G * KT], f32, isOutput=True)

    with tile.TileContext(nc) as tc:
        with (
            tc.tile_pool(name="desc", bufs=1) as desc_pool,
            tc.tile_pool(name="mask", bufs=2) as mask_pool,
            tc.tile_pool(name="scr", bufs=4) as scr_pool,
            tc.tile_pool(name="accs", bufs=1) as acc_pool,
            tc.tile_pool(name="psum", bufs=3, space="PSUM") as psum_pool,
        ):
            a_t = desc_pool.tile([D, ROWS_PER_CORE], bf16, tag="a")
            b_t = desc_pool.tile([D, IJ], bf16, tag="b")
            nc.sync.dma_start(a_t[:], a5[:])
            nc.sync.dma_start(b_t[:], bm[:])

            acc1_t = acc_pool.tile([128, G * KT], f32, tag="acc1")
            acc2_t = acc_pool.tile([128, G * KT], f32, tag="acc2")

            for g in range(G):
                xm_t = mask_pool.tile([128, IJ], f8, tag="xm")
                ym_t = mask_pool.tile([128, IJ], f8, tag="ym")
                rs = slice(g * 128, (g + 1) * 128)
                nc.sync.dma_start(xm_t[:], xm[rs, :])
                nc.sync.dma_start(ym_t[:], ym[rs, :])

                lhsT = a_t[:, rs]
                for k in range(KT):
                    t = g * KT + k
                    psum_d = psum_pool.tile([128, KTILE], f32, tag="d")
                    for h in range(KTILE // 512):
                        cs = slice(k * KTILE + h * 512, k * KTILE + (h + 1) * 512)
                        nc.tensor.matmul(
                            psum_d[:, h * 512:(h + 1) * 512],
                            lhsT,
                            b_t[:, cs],
                            start=True,
                            stop=True,
                        )
                    ks = slice(k * KTILE, (k + 1) * KTILE)
                    scr1 = scr_pool.tile([128, KTILE], bf16, tag="scr")
                    scr2 = scr_pool.tile([128, KTILE], bf16, tag="scr")
                    nc.vector.scalar_tensor_tensor(
                        scr1[:],
                        psum_d[:],
                        1.0,
                        xm_t[:, ks],
                        op0=mybir.AluOpType.max,
                        op1=mybir.AluOpType.min,
                        accum_out=acc1_t[:, t:t + 1],
                    )
                    nc.vector.scalar_tensor_tensor(
                        scr2[:],
                        psum_d[:],
                        5.0,
                        ym_t[:, ks],
                        op0=mybir.AluOpType.min,
                        op1=mybir.AluOpType.max,
                        accum_out=acc2_t[:, t:t + 1],
                    )

            nc.sync.dma_start(acc1_out[:], acc1_t[:])
            nc.sync.dma_start(acc2_out[:], acc2_t[:])

    nc.finalize()
    return nc


def _prep_inputs(descriptors_0, descriptors_1, similarity_mask):
    d0 = np.asarray(descriptors_0, dtype=np.float32)
    d1 = np.asarray(descriptors_1, dtype=np.float32)
    mk = np.asarray(similarity_mask)
    in_maps = []
    for c in range(N_CORES):
        b = c >> 2
        isl = (c & 3) * 16
        a5 = (d0[b].reshape(D, IJ)[:, isl * W:(isl + 16) * W] * np.float32(5.0)).astype(
            ml_dtypes.bfloat16
        )
        bm = d1[b].reshape(D, IJ).astype(ml_dtypes.bfloat16)
        m = mk[b, isl:isl + 16].reshape(ROWS_PER_CORE, IJ)
        xmv = np.where(m, np.float32(1.0), np.float32(CLAMP)).astype(
            ml_dtypes.float8_e5m2
        )
        ymv = np.where(m, np.float32(-CLAMP), np.float32(5.0)).astype(
            ml_dtypes.float8_e5m2
        )
        in_maps.append(
            {
                "a5": np.ascontiguousarray(a5),
                "bm": np.ascontiguousarray(bm),
                "xm": np.ascontiguousarray(xmv),
                "ym": np.ascontiguousarray(ymv),
            }
        )
    return in_maps


def _run(in_maps, **kwargs):
    if "nc" not in _cached:
        _cached["nc"] = _build_program()
    return run_bass_kernel_spmd(_cached["nc"], in_maps, list(range(N_CORES)), **kwargs)


def _combine(results):
    total = 0.0
    for r in results:
        acc1 = r["acc1"].astype(np.float64).sum()
        acc2 = r["acc2"].astype(np.float64).sum()
        total += (acc1 - 250.0 * acc2 + 1249.0 * N_PER_CORE) / 5.0
    return np.float32(total / float(B * IJ * IJ))


def kernel(descriptors_0, descriptors_1, similarity_mask):
    in_maps = _prep_inputs(descriptors_0, descriptors_1, similarity_mask)
    res = _run(in_maps)
    return _combine(res.results)
